# revision 50
# baseline (speedup 1.0000x reference)
"""Causal self-attention Trainium2 kernel (B=2, T=4096, C=768, H=12, D=64).

Sharding: 8 cores = 2 batches x 4 head-groups (3 heads each).
Each core computes, for its (batch b, heads h0..h2):
  - QKV projection from x[b].T (transposed + bf16-cast on host)
  - causal flash attention in score-transposed layout (S^T tiles [k=128, q=512])
  - output projection partial out_p = sum_h (O_h / l_h) @ Wout[h*64:(h+1)*64]
Host gathers: out[b] = sum of the 4 partials + bout.

v3: the per-supertile epilogue (softmax normalization + output projection)
is interleaved into the NEXT supertile's attention as PE filler units that
share the qkv PSUM slots, so the kernel has no serial tail; heads h0+h1 are
packed into a single K=128 out-proj matmul (h2 is K=64); V-projection
matmuls are 195 wide (no pad) and the softmax-denominator ones column is
memset once instead of injected by matmul.
"""

import numpy as np
from contextlib import ExitStack

import concourse.bass as bass
import concourse.bacc as bacc
import concourse.mybir as mybir
import concourse.tile as tile
from concourse.bass_utils import run_bass_kernel_spmd

B, T, C, H, D = 2, 4096, 768, 12, 64
NCORES = 8
HPC = 3  # heads per core
GPB = 4  # head-groups per batch
SCALE = float(np.sqrt(D))  # 8.0
QS = 512  # q supertile (columns of S^T tiles)
KT = 128  # k tile (partitions of S^T tiles)
NQS = T // QS  # 8
NCH = C // 128  # 6 contraction chunks
G = 2  # S^T tiles per exp batch
VW = HPC * 64 + HPC  # vo block: [V0|1|V1|1|V2|1] = 195 cols

F32 = mybir.dt.float32
BF16 = mybir.dt.bfloat16
AX = mybir.AxisListType
ALU = mybir.AluOpType
ACTF = mybir.ActivationFunctionType


def build_nc(with_qkv_bias: bool, repeat: int = 1, parts=('qkv', 'attn', 'proj')):
    nc = bacc.Bacc()

    xt = nc.dram_tensor("xt", [C, T], BF16, kind="ExternalInput")
    wqk = nc.dram_tensor("wqk", [128, NCH * 384], BF16, kind="ExternalInput")
    wqk_b = nc.dram_tensor("wqk_b", [1, 384], BF16, kind="ExternalInput")
    wv = nc.dram_tensor("wv", [128, NCH * VW], BF16, kind="ExternalInput")
    wv_b = nc.dram_tensor("wv_b", [1, VW], BF16, kind="ExternalInput")
    wout = nc.dram_tensor("wout", [HPC * 64, C], BF16, kind="ExternalInput")
    trid = nc.dram_tensor("trid", [128, 128], BF16, kind="ExternalInput")
    ones_d = nc.dram_tensor("ones_d", [1, QS], BF16, kind="ExternalInput")
    out_p = nc.dram_tensor("out_p", [T, C], BF16, kind="ExternalOutput")

    do_qkv = 'qkv' in parts
    do_attn = 'attn' in parts
    do_proj = 'proj' in parts
    k_exp = 'noexp' not in parts
    k_mask = 'nomask' not in parts
    k_pv = 'nopv' not in parts
    k_norm = 'nonorm' not in parts

    with tile.TileContext(nc) as tc, ExitStack() as ctx:
        # weights/constants load once, outside the repeat loop
        const = ctx.enter_context(tc.tile_pool(name="const", bufs=1))

        wqk_sb = const.tile([128, NCH * 384], BF16, tag="wqk")
        nc.sync.dma_start(wqk_sb[:], wqk[:])
        wv_sb = const.tile([128, NCH * VW], BF16, tag="wv")
        nc.sync.dma_start(wv_sb[:], wv[:])
        wvb_sb = const.tile([1, VW], BF16, tag="wvb")
        wqkb_sb = const.tile([1, 384], BF16, tag="wqkb")
        if with_qkv_bias:
            nc.sync.dma_start(wqkb_sb[:], wqk_b[:])
            nc.sync.dma_start(wvb_sb[:], wv_b[:])
        tri_sb = const.tile([128, 128], BF16, tag="tri")
        nc.sync.dma_start(tri_sb[:], trid[:])
        wout01_sb = const.tile([128, C], BF16, tag="wout01")
        nc.sync.dma_start(wout01_sb[:], wout[0:128, :])
        wout2_sb = const.tile([64, C], BF16, tag="wout2")
        nc.sync.dma_start(wout2_sb[:], wout[128:192, :])

        ones_row = const.tile([1, QS], BF16, tag="ones_row")
        nc.sync.dma_start(ones_row[:], ones_d[:])
        # ones column on partitions 0-64 (rank-1 lhsT at base 0/32/64)
        ones65c = const.tile([65, 64], BF16, tag="ones65c")
        nc.vector.memset(ones65c[:], 1.0)

        qt01 = const.tile([128, T], BF16, tag="qt01")
        kt01 = const.tile([128, T], BF16, tag="kt01")
        qkt2 = const.tile([128, T], BF16, tag="qkt2")
        dup2 = const.tile([128, T], BF16, tag="dup2")
        vo = const.tile([128, (T // 128) * VW], BF16, tag="vo")
        # O^T accumulators: h0 rows 0-63, h1 rows 64-127; h2 separate
        ot01 = const.tile([128, T], BF16, tag="ot01")
        ot2 = const.tile([64, T], BF16, tag="ot2")

        # softmax-denominator ones columns (never touched by the v copies)
        vo_ones = vo.rearrange("p (t c) -> p t c", c=65)[:, :, 64:65]
        nc.vector.memset(vo_ones, 1.0)

        rep_scope = ExitStack()
        if repeat > 1:
            rep_scope.enter_context(tc.For_i(0, repeat, 1))

        main_scope = ExitStack()
        xpool = main_scope.enter_context(tc.tile_pool(name="xt", bufs=3))
        qkv_ps = main_scope.enter_context(tc.tile_pool(name="qkvps", bufs=2, space="PSUM"))
        sg_ps = main_scope.enter_context(tc.tile_pool(name="sgps", bufs=2, space="PSUM"))
        ot_ps = main_scope.enter_context(tc.tile_pool(name="otps", bufs=2, space="PSUM"))
        ppool = main_scope.enter_context(tc.tile_pool(name="pt", bufs=4))
        rpool = main_scope.enter_context(tc.tile_pool(name="rsb", bufs=24))
        opool = main_scope.enter_context(tc.tile_pool(name="osb", bufs=2))

        def qkv_units(it):
            """Emit x DMAs now; return PE work-unit closures to interleave."""
            tw = slice(it * QS, (it + 1) * QS)
            xts = []
            for c in range(NCH):
                xtile = xpool.tile([128, QS], BF16, tag=f"x{c % 3}", name=f"xt{it}_{c}")
                nc.sync.dma_start(xtile[:], xt[c * 128:(c + 1) * 128, tw])
                xts.append(xtile)
            units = []

            def pack_unit(p, dest):
                def emit():
                    ps = qkv_ps.tile([128, QS], F32, tag="qkv", name=f"qk{it}_{p}")
                    for c in range(NCH):
                        nc.tensor.matmul(
                            ps[:],
                            wqk_sb[:, c * 384 + p * 128: c * 384 + (p + 1) * 128],
                            xts[c][:],
                            start=(c == 0),
                            stop=(not with_qkv_bias and c == NCH - 1),
                        )
                    if with_qkv_bias:
                        nc.tensor.matmul(
                            ps[:], wqkb_sb[:, p * 128:(p + 1) * 128], ones_row[:],
                            start=False, stop=True,
                        )
                    with nc.allow_low_precision(reason="bf16 matmul operand"):
                        nc.vector.tensor_copy(dest[:, tw], ps[:])
                    if p == 2:
                        nc.sync.dma_start(dup2[0:64, tw], qkt2[64:128, tw])
                        nc.sync.dma_start(dup2[64:128, tw], qkt2[0:64, tw])
                return emit

            def v_unit(st):
                def emit():
                    tcn = it * (QS // 128) + st
                    ps = qkv_ps.tile([128, VW], F32, tag="qkv", name=f"v{it}_{st}")
                    for c in range(NCH):
                        nc.tensor.matmul(
                            ps[:],
                            xts[c][:, st * 128:(st + 1) * 128],
                            wv_sb[:, c * VW:(c + 1) * VW],
                            start=(c == 0),
                            stop=(not with_qkv_bias and c == NCH - 1),
                        )
                    if with_qkv_bias:
                        # bias (ones-col values unused: copy below skips them)
                        nc.tensor.matmul(
                            ps[:], ones_row[:, st * 128:(st + 1) * 128], wvb_sb[:],
                            start=False, stop=True,
                        )
                    # copy only the 3x64 V blocks; ones cols stay memset
                    ps_v = ps.rearrange("p (g c) -> p g c", c=65)[:, :, 0:64]
                    vo_v = vo[:, tcn * VW:(tcn + 1) * VW] \
                        .rearrange("p (g c) -> p g c", c=65)[:, :, 0:64]
                    with nc.allow_low_precision(reason="bf16 matmul operand"):
                        nc.vector.tensor_copy(vo_v, ps_v)
                return emit

            for p, dest in ((0, qt01), (1, kt01), (2, qkt2)):
                units.append(pack_unit(p, dest))
            for st in range(QS // 128):
                units.append(v_unit(st))
            return units

        # h -> (O^T dest tile, row offset)
        ot_dest = {0: (ot01, 0), 1: (ot01, 64), 2: (ot2, 0)}
        # per-supertile denominator tiles (heads on PE-aligned rows 0/32/64);
        # epilogues may run several supertiles later
        L3ROW = {0: 0, 1: 32, 2: 64}
        l3_all = {}
        rsb3_all = {}

        def attend(s, jobs, work_q=None):
            """jobs: list of (h, ktb_fn, qtb_fn)."""
            qw = slice(s * QS, (s + 1) * QS)
            nk = 4 * (s + 1)
            otps = {}
            for h, _, _ in jobs:
                otps[h] = ot_ps.tile([65, QS], F32, tag="ot", name=f"otp{h}_{s}")

            def mstart(jj):  # first unmasked col (within the 512-q window)
                m = jj - (nk - 4)
                return m * 128 if m > 0 else 0

            def emit_pv(pts, g0, gn):
                for h, _, _ in jobs:
                    for jl in range(gn):
                        jj = g0 + jl
                        c0 = mstart(jj)
                        m = jj - (nk - 4)
                        if k_mask and m >= 0:
                            # triangle block only: q-subtile == k-tile index
                            blk = slice(jl * QS + c0, jl * QS + c0 + 128)
                            nc.vector.tensor_tensor(
                                pts[h][:, blk], pts[h][:, blk], tri_sb[:],
                                op=ALU.mult,
                            )
                        if k_pv:
                            nc.tensor.matmul(
                                otps[h][:, c0:QS],
                                vo[:, jj * VW + h * 65: jj * VW + (h + 1) * 65],
                                pts[h][:, jl * QS + c0:(jl + 1) * QS],
                                start=(jj == 0),
                                stop=(jj == nk - 1),
                                skip_group_check=True,
                            )

            prev = None
            for g0 in range(0, nk, G):
                gn = min(G, nk - g0)
                sgps, pts = {}, {}
                for h, ktb_fn, qtb_fn in jobs:
                    sgps[h] = sg_ps.tile([128, G * QS], F32, tag="sg",
                                         name=f"sg{h}_{s}_{g0}")
                for jl in range(gn):
                    jj = g0 + jl
                    c0 = mstart(jj)
                    for h, ktb_fn, qtb_fn in jobs:
                        nc.tensor.matmul(
                            sgps[h][:, jl * QS + c0:(jl + 1) * QS],
                            ktb_fn(jj),
                            qtb_fn(jj)[:, s * QS + c0:(s + 1) * QS],
                            start=True, stop=True,
                        )
                # PV for the previous group fills PE while ACT exps this group
                if prev is not None:
                    emit_pv(*prev)
                if work_q:
                    work_q.pop(0)()
                for h, _, _ in jobs:
                    pt = ppool.tile([128, G * QS], BF16, tag="pt",
                                    name=f"pt{h}_{s}_{g0}")
                    if k_exp:
                        # fuse contiguous full tiles into one ACT instr
                        runs = []
                        for jl in range(gn):
                            c0 = mstart(g0 + jl)
                            lo, hi = jl * QS + c0, (jl + 1) * QS
                            if runs and runs[-1][1] == lo and c0 == 0:
                                runs[-1][1] = hi
                            else:
                                runs.append([lo, hi])
                        for lo, hi in runs:
                            nc.scalar.activation(
                                pt[:, lo:hi], sgps[h][:, lo:hi], ACTF.Exp,
                                scale=1.0 / SCALE,
                            )
                    pts[h] = pt
                prev = (pts, g0, gn)
            if prev is not None:
                emit_pv(*prev)
            for h, _, _ in jobs:
                if not k_pv:
                    continue
                dest, r0 = ot_dest[h]
                with nc.allow_low_precision(reason="bf16 matmul operand"):
                    nc.vector.tensor_copy(dest[r0:r0 + 64, qw], otps[h][0:64, :])
                if k_norm:
                    # drain only a cheap copy off the otps slot; the (slow,
                    # iterative) reciprocal runs deferred, batched per
                    # supertile, fully overlapped with later attention
                    if s not in l3_all:
                        l3 = rpool.tile([65, QS], F32, tag="l3", bufs=6,
                                        name=f"l3_{s}")
                        nc.vector.memset(l3[:], 1.0)
                        l3_all[s] = l3
                    nc.vector.tensor_copy(
                        l3_all[s][L3ROW[h]:L3ROW[h] + 1, :],
                        otps[h][64:65, :])

        def recip_unit(s):
            def emit():
                # one batched reciprocal covers all 3 heads (free-dim bound)
                rsb3 = rpool.tile([65, QS], BF16, tag="rs", bufs=6,
                                  name=f"rs3_{s}")
                with nc.allow_low_precision(reason="bf16 recip"):
                    nc.vector.reciprocal(rsb3[:], l3_all[s][:])
                rsb3_all[s] = rsb3
            return emit

        def norm_unit(h, s):
            def emit():
                qw = slice(s * QS, (s + 1) * QS)
                dest, r0 = ot_dest[h]
                # broadcast 1/l across partitions via a rank-1 PE matmul
                # (GpSimd partition_broadcast is far slower on real HW than
                # the sim's cost model suggests)
                r = L3ROW[h]
                rp = qkv_ps.tile([64, QS], F32, tag="qkv", name=f"rp{h}_{s}")
                nc.tensor.matmul(
                    rp[:], ones65c[r:r + 1, :], rsb3_all[s][r:r + 1, :],
                    start=True, stop=True,
                )
                with nc.allow_low_precision(reason="bf16 norm"):
                    nc.vector.tensor_tensor(
                        dest[r0:r0 + 64, qw], dest[r0:r0 + 64, qw], rp[:],
                        op=ALU.mult,
                    )
            return emit

        def proj_unit(tcn, pools=None):
            def emit():
                tw = slice(tcn * 128, (tcn + 1) * 128)
                osb = opool.tile([128, C], BF16, tag="osb", name=f"osb{tcn}")
                for half in range(2):
                    cw = slice(half * 384, (half + 1) * 384)
                    pool, tag = (pools[half % len(pools)] if pools
                                 else (qkv_ps, "qkv"))
                    ps = pool.tile([128, 384], F32, tag=tag,
                                   name=f"op{tcn}_{half}")
                    nc.tensor.matmul(
                        ps[:], ot01[:, tw], wout01_sb[:, cw],
                        start=True, stop=False,
                    )
                    nc.tensor.matmul(
                        ps[:], ot2[:, tw], wout2_sb[:, cw],
                        start=False, stop=True,
                    )
                    with nc.allow_low_precision(reason="bf16 out"):
                        nc.vector.tensor_copy(osb[:, cw], ps[:])
                nc.sync.dma_start(out_p[tw, :], osb[:])
            return emit

        def epilogue_units(s, tail=False):
            """Normalize + project supertile s (deferrable past attend(s))."""
            units = []
            if k_norm:
                units.append(recip_unit(s))
                for h in range(HPC):
                    units.append(norm_unit(h, s))
            if do_proj:
                # tail: attention PSUM pools are free; spread the proj tiles
                # across them so slot-drain (DVE copy) never stalls PE
                pools = [(sg_ps, "sg"), (ot_ps, "ot")] if tail else None
                for st in range(QS // 128):
                    units.append(proj_unit(s * (QS // 128) + st, pools))
            return units

        heads = (
            (kt01[0:64, :], qt01[0:64, :]),
            (kt01[64:128, :], qt01[64:128, :]),
        )

        def attend_all(s, work_q):
            attend(s, [
                (0, lambda jj: heads[0][0][:, jj * KT:(jj + 1) * KT],
                    lambda jj: heads[0][1]),
                (1, lambda jj: heads[1][0][:, jj * KT:(jj + 1) * KT],
                    lambda jj: heads[1][1]),
            ], work_q)

            attend(s, [
                (2, lambda jj: (dup2[0:64, jj * KT:(jj + 1) * KT] if jj % 2 == 0
                                else qkt2[64:128, jj * KT:(jj + 1) * KT]),
                    lambda jj: (qkt2[0:64, :] if jj % 2 == 0 else dup2[64:128, :])),
            ], work_q)
            # leftovers (small s): emit before the next q_super needs them
            while work_q:
                work_q.pop(0)()

        # software-pipelined: qkv(it+1) units + deferred epilogue units
        # interleaved into attend(it). Epilogues are pushed into the LATE
        # (ACT-bound) supertiles where PE has idle slots.
        EPI_SCHED = {4: [0], 5: [1, 2], 6: [3, 4], 7: [5, 6]}
        EPI_TAIL = [7]
        if not do_attn or not k_pv:
            EPI_SCHED, EPI_TAIL = {}, []
        if do_qkv:
            for u in qkv_units(0):
                u()
        for it in range(NQS):
            work_q = []
            for es in EPI_SCHED.get(it, []):
                work_q += epilogue_units(es)
            if do_qkv and it + 1 < NQS:
                work_q += qkv_units(it + 1)
            if do_attn:
                attend_all(it, work_q)
            else:
                while work_q:
                    work_q.pop(0)()
        for es in EPI_TAIL:
            for u in epilogue_units(es, tail=True):
                u()

        main_scope.close()

        if not do_proj:
            nc.sync.dma_start(out_p[0:128, :], qt01[:, 0:C])

        rep_scope.close()

    nc.compile()
    return nc


_NC_CACHE = {}


def _get_nc(with_qkv_bias: bool, repeat: int = 1, parts=('qkv', 'attn', 'proj')):
    key = (with_qkv_bias, repeat, tuple(parts))
    if key not in _NC_CACHE:
        _NC_CACHE[key] = build_nc(with_qkv_bias, repeat, parts)
    return _NC_CACHE[key]


def _prep_inputs(x, Wqkv, bqkv, Wout, bout):
    """Build the 8 per-core input maps (bf16 host-side casts)."""
    BF = mybir.dt.np(BF16)
    x = np.asarray(x, dtype=np.float32)
    Wqkv = np.asarray(Wqkv, dtype=np.float32)
    bqkv = np.asarray(bqkv, dtype=np.float32)
    Wout = np.asarray(Wout, dtype=np.float32)

    with_qkv_bias = bool(np.any(bqkv))

    # triangular block mask: tri[kk, qq] = qq >= kk
    kk = np.arange(128)[:, None]
    qq = np.arange(128)[None, :]
    tri = (qq >= kk).astype(np.float32)

    xts = [np.ascontiguousarray(x[b].T).astype(BF) for b in range(B)]  # [C, T]

    in_maps = []
    for core in range(NCORES):
        b = core // GPB
        hs = [(core % GPB) * HPC + i for i in range(HPC)]  # 3 head indices

        def col(i, h):  # Wqkv column block for (q/k/v i, head h)
            return Wqkv[:, i * C + h * D: i * C + (h + 1) * D]

        def bias(i, h):
            return bqkv[i * C + h * D: i * C + (h + 1) * D]

        # packs: [q0|q1], [k0|k1], [q2|k2]
        wqk = np.concatenate(
            [col(0, hs[0]), col(0, hs[1]),
             col(1, hs[0]), col(1, hs[1]),
             col(0, hs[2]), col(1, hs[2])], axis=1,
        )  # [768, 384]
        wqk_c = wqk.reshape(NCH, 128, 384).transpose(1, 0, 2).reshape(128, NCH * 384)
        wqk_b = np.concatenate(
            [bias(0, hs[0]), bias(0, hs[1]),
             bias(1, hs[0]), bias(1, hs[1]),
             bias(0, hs[2]), bias(1, hs[2])]
        ).reshape(1, 384)

        # V: blocks of 65 cols per head [v_h | 0]; ones col comes from memset
        wv = np.zeros((C, VW), dtype=np.float32)
        wv_b = np.zeros((1, VW), dtype=np.float32)
        for i, h in enumerate(hs):
            wv[:, i * 65: i * 65 + 64] = col(2, h)
            wv_b[0, i * 65: i * 65 + 64] = bias(2, h)
        wv_c = wv.reshape(NCH, 128, VW).transpose(1, 0, 2).reshape(128, NCH * VW)

        # out-proj rows: h0 0-63, h1 64-127, h2 128-191
        wout_c = np.concatenate(
            [Wout[h * D:(h + 1) * D, :] for h in hs], axis=0)  # [192, 768]

        in_maps.append({
            "xt": np.ascontiguousarray(xts[b]),
            "wqk": np.ascontiguousarray(wqk_c).astype(BF),
            "wqk_b": np.ascontiguousarray(wqk_b).astype(BF),
            "wv": np.ascontiguousarray(wv_c).astype(BF),
            "wv_b": np.ascontiguousarray(wv_b).astype(BF),
            "wout": np.ascontiguousarray(wout_c).astype(BF),
            "trid": tri.astype(BF),
            "ones_d": np.ones((1, QS), dtype=np.float32).astype(BF),
        })
    return in_maps, with_qkv_bias


def kernel(x, Wqkv, bqkv, Wout, bout, _trace=False, _trace_kwargs=None, _repeat=1,
           _parts=('qkv', 'attn', 'proj')):
    in_maps, with_qkv_bias = _prep_inputs(x, Wqkv, bqkv, Wout, bout)
    nc = _get_nc(with_qkv_bias, _repeat, _parts)
    res = run_bass_kernel_spmd(
        nc, in_maps, list(range(NCORES)), trace=_trace,
        **(_trace_kwargs or {}),
    )
    bout = np.asarray(bout, dtype=np.float32)
    parts = np.stack([res.results[i]["out_p"].astype(np.float32)
                      for i in range(NCORES)])
    out = parts.reshape(B, GPB, T, C).sum(axis=1) + bout
    kernel._last_result = res
    return out.astype(np.float32)


# revision 52
# speedup vs baseline: 1.0585x; 1.0585x over previous
"""Causal self-attention Trainium2 kernel (B=2, T=4096, C=768, H=12, D=64).

Sharding: 8 cores = 2 batches x 4 head-groups (3 heads each).
Each core computes, for its (batch b, heads h0..h2):
  - QKV projection from x[b].T (transposed + bf16-cast on host)
  - causal flash attention in score-transposed layout (S^T tiles [k=128, q=512])
  - output projection partial out_p = sum_h (O_h / l_h) @ Wout[h*64:(h+1)*64]
Host gathers: out[b] = sum of the 4 partials + bout.

v3: the per-supertile epilogue (softmax normalization + output projection)
is interleaved into the NEXT supertile's attention as PE filler units that
share the qkv PSUM slots, so the kernel has no serial tail; heads h0+h1 are
packed into a single K=128 out-proj matmul (h2 is K=64); V-projection
matmuls are 195 wide (no pad) and the softmax-denominator ones column is
memset once instead of injected by matmul.
"""

import numpy as np
from contextlib import ExitStack

import concourse.bass as bass
import concourse.bacc as bacc
import concourse.mybir as mybir
import concourse.tile as tile
from concourse.bass_utils import run_bass_kernel_spmd

B, T, C, H, D = 2, 4096, 768, 12, 64
NCORES = 8
HPC = 3  # heads per core
GPB = 4  # head-groups per batch
SCALE = float(np.sqrt(D))  # 8.0
QS = 512  # q supertile (columns of S^T tiles)
KT = 128  # k tile (partitions of S^T tiles)
NQS = T // QS  # 8
NCH = C // 128  # 6 contraction chunks
G = 2  # S^T tiles per exp batch
VW = HPC * 64 + HPC  # vo block: [V0|1|V1|1|V2|1] = 195 cols

F32 = mybir.dt.float32
BF16 = mybir.dt.bfloat16
AX = mybir.AxisListType
ALU = mybir.AluOpType
ACTF = mybir.ActivationFunctionType


def build_nc(with_qkv_bias: bool, repeat: int = 1, parts=('qkv', 'attn', 'proj')):
    nc = bacc.Bacc()

    xt = nc.dram_tensor("xt", [C, T], BF16, kind="ExternalInput")
    wqk = nc.dram_tensor("wqk", [128, NCH * 384], BF16, kind="ExternalInput")
    wqk_b = nc.dram_tensor("wqk_b", [1, 384], BF16, kind="ExternalInput")
    wv = nc.dram_tensor("wv", [128, NCH * VW], BF16, kind="ExternalInput")
    wv_b = nc.dram_tensor("wv_b", [1, VW], BF16, kind="ExternalInput")
    wout = nc.dram_tensor("wout", [HPC * 64, C], BF16, kind="ExternalInput")
    trid = nc.dram_tensor("trid", [128, 128], BF16, kind="ExternalInput")
    ones_d = nc.dram_tensor("ones_d", [1, QS], BF16, kind="ExternalInput")
    out_p = nc.dram_tensor("out_p", [T, C], BF16, kind="ExternalOutput")

    do_qkv = 'qkv' in parts
    do_attn = 'attn' in parts
    do_proj = 'proj' in parts
    k_exp = 'noexp' not in parts
    k_mask = 'nomask' not in parts
    k_pv = 'nopv' not in parts
    k_norm = 'nonorm' not in parts

    with tile.TileContext(nc) as tc, ExitStack() as ctx:
        # weights/constants load once, outside the repeat loop
        const = ctx.enter_context(tc.tile_pool(name="const", bufs=1))

        wqk_sb = const.tile([128, NCH * 384], BF16, tag="wqk")
        nc.sync.dma_start(wqk_sb[:], wqk[:])
        wv_sb = const.tile([128, NCH * VW], BF16, tag="wv")
        nc.sync.dma_start(wv_sb[:], wv[:])
        wvb_sb = const.tile([1, VW], BF16, tag="wvb")
        wqkb_sb = const.tile([1, 384], BF16, tag="wqkb")
        if with_qkv_bias:
            nc.sync.dma_start(wqkb_sb[:], wqk_b[:])
            nc.sync.dma_start(wvb_sb[:], wv_b[:])
        tri_sb = const.tile([128, 128], BF16, tag="tri")
        nc.sync.dma_start(tri_sb[:], trid[:])
        wout01_sb = const.tile([128, C], BF16, tag="wout01")
        nc.sync.dma_start(wout01_sb[:], wout[0:128, :])
        wout2_sb = const.tile([64, C], BF16, tag="wout2")
        nc.sync.dma_start(wout2_sb[:], wout[128:192, :])

        ones_row = const.tile([1, QS], BF16, tag="ones_row")
        nc.sync.dma_start(ones_row[:], ones_d[:])
        # ones column on partitions 0-64 (rank-1 lhsT at base 0/32/64)
        ones65c = const.tile([65, 64], BF16, tag="ones65c")
        nc.vector.memset(ones65c[:], 1.0)

        qt01 = const.tile([128, T], BF16, tag="qt01")
        kt01 = const.tile([128, T], BF16, tag="kt01")
        qkt2 = const.tile([128, T], BF16, tag="qkt2")
        dup2 = const.tile([128, T], BF16, tag="dup2")
        vo = const.tile([128, (T // 128) * VW], BF16, tag="vo")
        # O^T accumulators: h0 rows 0-63, h1 rows 64-127; h2 separate
        ot01 = const.tile([128, T], BF16, tag="ot01")
        ot2 = const.tile([64, T], BF16, tag="ot2")

        # softmax-denominator ones columns (never touched by the v copies)
        vo_ones = vo.rearrange("p (t c) -> p t c", c=65)[:, :, 64:65]
        nc.vector.memset(vo_ones, 1.0)

        rep_scope = ExitStack()
        if repeat > 1:
            rep_scope.enter_context(tc.For_i(0, repeat, 1))

        main_scope = ExitStack()
        xpool = main_scope.enter_context(tc.tile_pool(name="xt", bufs=3))
        qkv_ps = main_scope.enter_context(tc.tile_pool(name="qkvps", bufs=2, space="PSUM"))
        sg_ps = main_scope.enter_context(tc.tile_pool(name="sgps", bufs=2, space="PSUM"))
        ot_ps = main_scope.enter_context(tc.tile_pool(name="otps", bufs=2, space="PSUM"))
        ppool = main_scope.enter_context(tc.tile_pool(name="pt", bufs=4))
        rpool = main_scope.enter_context(tc.tile_pool(name="rsb", bufs=24))
        opool = main_scope.enter_context(tc.tile_pool(name="osb", bufs=2))

        def qkv_units(it):
            """Emit x DMAs now; return PE work-unit closures to interleave."""
            tw = slice(it * QS, (it + 1) * QS)
            xts = []
            for c in range(NCH):
                xtile = xpool.tile([128, QS], BF16, tag=f"x{c % 3}", name=f"xt{it}_{c}")
                nc.sync.dma_start(xtile[:], xt[c * 128:(c + 1) * 128, tw])
                xts.append(xtile)
            units = []

            def pack_unit(p, dest):
                def emit():
                    ps = qkv_ps.tile([128, QS], F32, tag="qkv", name=f"qk{it}_{p}")
                    for c in range(NCH):
                        nc.tensor.matmul(
                            ps[:],
                            wqk_sb[:, c * 384 + p * 128: c * 384 + (p + 1) * 128],
                            xts[c][:],
                            start=(c == 0),
                            stop=(not with_qkv_bias and c == NCH - 1),
                        )
                    if with_qkv_bias:
                        nc.tensor.matmul(
                            ps[:], wqkb_sb[:, p * 128:(p + 1) * 128], ones_row[:],
                            start=False, stop=True,
                        )
                    with nc.allow_low_precision(reason="bf16 matmul operand"):
                        nc.vector.tensor_copy(dest[:, tw], ps[:])
                    if p == 2:
                        nc.sync.dma_start(dup2[0:64, tw], qkt2[64:128, tw])
                        nc.sync.dma_start(dup2[64:128, tw], qkt2[0:64, tw])
                return emit

            def v_unit(st):
                def emit():
                    tcn = it * (QS // 128) + st
                    ps = qkv_ps.tile([128, VW], F32, tag="qkv", name=f"v{it}_{st}")
                    for c in range(NCH):
                        nc.tensor.matmul(
                            ps[:],
                            xts[c][:, st * 128:(st + 1) * 128],
                            wv_sb[:, c * VW:(c + 1) * VW],
                            start=(c == 0),
                            stop=(not with_qkv_bias and c == NCH - 1),
                        )
                    if with_qkv_bias:
                        # bias (ones-col values unused: copy below skips them)
                        nc.tensor.matmul(
                            ps[:], ones_row[:, st * 128:(st + 1) * 128], wvb_sb[:],
                            start=False, stop=True,
                        )
                    # copy only the 3x64 V blocks; ones cols stay memset
                    ps_v = ps.rearrange("p (g c) -> p g c", c=65)[:, :, 0:64]
                    vo_v = vo[:, tcn * VW:(tcn + 1) * VW] \
                        .rearrange("p (g c) -> p g c", c=65)[:, :, 0:64]
                    with nc.allow_low_precision(reason="bf16 matmul operand"):
                        nc.vector.tensor_copy(vo_v, ps_v)
                return emit

            for p, dest in ((0, qt01), (1, kt01), (2, qkt2)):
                units.append(pack_unit(p, dest))
            for st in range(QS // 128):
                units.append(v_unit(st))
            return units

        # h -> (O^T dest tile, row offset)
        ot_dest = {0: (ot01, 0), 1: (ot01, 64), 2: (ot2, 0)}
        # per-supertile denominator tiles (heads on PE-aligned rows 0/32/64);
        # epilogues may run several supertiles later
        L3ROW = {0: 0, 1: 32, 2: 64}
        l3_all = {}
        rsb3_all = {}

        def attend(s, jobs, work_q=None):
            """jobs: list of (h, ktb_fn, qtb_fn)."""
            qw = slice(s * QS, (s + 1) * QS)
            nk = 4 * (s + 1)
            otps = {}
            for h, _, _ in jobs:
                otps[h] = ot_ps.tile([65, QS], F32, tag="ot", name=f"otp{h}_{s}")

            def mstart(jj):  # first unmasked col (within the 512-q window)
                m = jj - (nk - 4)
                return m * 128 if m > 0 else 0

            def emit_pv(pts, g0, gn):
                for h, _, _ in jobs:
                    for jl in range(gn):
                        jj = g0 + jl
                        c0 = mstart(jj)
                        m = jj - (nk - 4)
                        if k_mask and m >= 0:
                            # triangle block only: q-subtile == k-tile index
                            blk = slice(jl * QS + c0, jl * QS + c0 + 128)
                            nc.vector.tensor_tensor(
                                pts[h][:, blk], pts[h][:, blk], tri_sb[:],
                                op=ALU.mult,
                            )
                        if k_pv:
                            nc.tensor.matmul(
                                otps[h][:, c0:QS],
                                vo[:, jj * VW + h * 65: jj * VW + (h + 1) * 65],
                                pts[h][:, jl * QS + c0:(jl + 1) * QS],
                                start=(jj == 0),
                                stop=(jj == nk - 1),
                                skip_group_check=True,
                            )

            prev = None
            for g0 in range(0, nk, G):
                gn = min(G, nk - g0)
                sgps, pts = {}, {}
                for h, ktb_fn, qtb_fn in jobs:
                    sgps[h] = sg_ps.tile([128, G * QS], F32, tag="sg",
                                         name=f"sg{h}_{s}_{g0}")
                for jl in range(gn):
                    jj = g0 + jl
                    c0 = mstart(jj)
                    for h, ktb_fn, qtb_fn in jobs:
                        nc.tensor.matmul(
                            sgps[h][:, jl * QS + c0:(jl + 1) * QS],
                            ktb_fn(jj),
                            qtb_fn(jj)[:, s * QS + c0:(s + 1) * QS],
                            start=True, stop=True,
                        )
                # PV for the previous group fills PE while ACT exps this group
                if prev is not None:
                    emit_pv(*prev)
                if work_q:
                    work_q.pop(0)()
                for h, _, _ in jobs:
                    pt = ppool.tile([128, G * QS], BF16, tag="pt",
                                    name=f"pt{h}_{s}_{g0}")
                    if k_exp:
                        # fuse contiguous full tiles into one ACT instr
                        runs = []
                        for jl in range(gn):
                            c0 = mstart(g0 + jl)
                            lo, hi = jl * QS + c0, (jl + 1) * QS
                            if runs and runs[-1][1] == lo and c0 == 0:
                                runs[-1][1] = hi
                            else:
                                runs.append([lo, hi])
                        for lo, hi in runs:
                            nc.scalar.activation(
                                pt[:, lo:hi], sgps[h][:, lo:hi], ACTF.Exp,
                                scale=1.0 / SCALE,
                            )
                    pts[h] = pt
                prev = (pts, g0, gn)
            if prev is not None:
                emit_pv(*prev)
            for h, _, _ in jobs:
                if not k_pv:
                    continue
                dest, r0 = ot_dest[h]
                with nc.allow_low_precision(reason="bf16 matmul operand"):
                    nc.vector.tensor_copy(dest[r0:r0 + 64, qw], otps[h][0:64, :])
                if k_norm:
                    # drain only a cheap copy off the otps slot; the (slow,
                    # iterative) reciprocal runs deferred, batched per
                    # supertile, fully overlapped with later attention
                    if s not in l3_all:
                        l3 = rpool.tile([65, QS], F32, tag="l3", bufs=6,
                                        name=f"l3_{s}")
                        nc.vector.memset(l3[:], 1.0)
                        l3_all[s] = l3
                    nc.vector.tensor_copy(
                        l3_all[s][L3ROW[h]:L3ROW[h] + 1, :],
                        otps[h][64:65, :])

        def recip_units(s):
            # batched reciprocal covers all 3 heads (free-dim bound), but in
            # 128-col chunks: a monolithic 3.7us recip would head-of-line
            # block chain-critical DVE work (mask/copies) behind it
            def chunk(ci):
                def emit():
                    if s not in rsb3_all:
                        rsb3_all[s] = rpool.tile([65, QS], BF16, tag="rs",
                                                 bufs=6, name=f"rs3_{s}")
                    cw = slice(ci * 128, (ci + 1) * 128)
                    with nc.allow_low_precision(reason="bf16 recip"):
                        nc.vector.reciprocal(rsb3_all[s][:, cw],
                                             l3_all[s][:, cw])
                return emit
            return [chunk(ci) for ci in range(QS // 128)]

        def norm_unit(h, s):
            def emit():
                qw = slice(s * QS, (s + 1) * QS)
                dest, r0 = ot_dest[h]
                # broadcast 1/l across partitions via a rank-1 PE matmul
                # (GpSimd partition_broadcast is far slower on real HW than
                # the sim's cost model suggests)
                r = L3ROW[h]
                rp = qkv_ps.tile([64, QS], F32, tag="qkv", name=f"rp{h}_{s}")
                nc.tensor.matmul(
                    rp[:], ones65c[r:r + 1, :], rsb3_all[s][r:r + 1, :],
                    start=True, stop=True,
                )
                with nc.allow_low_precision(reason="bf16 norm"):
                    nc.vector.tensor_tensor(
                        dest[r0:r0 + 64, qw], dest[r0:r0 + 64, qw], rp[:],
                        op=ALU.mult,
                    )
            return emit

        def proj_unit(tcn, pools=None):
            def emit():
                tw = slice(tcn * 128, (tcn + 1) * 128)
                osb = opool.tile([128, C], BF16, tag="osb", name=f"osb{tcn}")
                for half in range(2):
                    cw = slice(half * 384, (half + 1) * 384)
                    pool, tag = (pools[half % len(pools)] if pools
                                 else (qkv_ps, "qkv"))
                    ps = pool.tile([128, 384], F32, tag=tag,
                                   name=f"op{tcn}_{half}")
                    nc.tensor.matmul(
                        ps[:], ot01[:, tw], wout01_sb[:, cw],
                        start=True, stop=False,
                    )
                    nc.tensor.matmul(
                        ps[:], ot2[:, tw], wout2_sb[:, cw],
                        start=False, stop=True,
                    )
                    with nc.allow_low_precision(reason="bf16 out"):
                        nc.vector.tensor_copy(osb[:, cw], ps[:])
                nc.sync.dma_start(out_p[tw, :], osb[:])
            return emit

        def epilogue_units(s, tail=False):
            """Normalize + project supertile s (deferrable past attend(s))."""
            units = []
            if k_norm:
                units.extend(recip_units(s))
                for h in range(HPC):
                    units.append(norm_unit(h, s))
            if do_proj:
                # tail: attention PSUM pools are free; spread the proj tiles
                # across them so slot-drain (DVE copy) never stalls PE
                pools = [(sg_ps, "sg"), (ot_ps, "ot")] if tail else None
                for st in range(QS // 128):
                    units.append(proj_unit(s * (QS // 128) + st, pools))
            return units

        heads = (
            (kt01[0:64, :], qt01[0:64, :]),
            (kt01[64:128, :], qt01[64:128, :]),
        )

        def attend_all(s, work_q):
            attend(s, [
                (0, lambda jj: heads[0][0][:, jj * KT:(jj + 1) * KT],
                    lambda jj: heads[0][1]),
                (1, lambda jj: heads[1][0][:, jj * KT:(jj + 1) * KT],
                    lambda jj: heads[1][1]),
            ], work_q)

            attend(s, [
                (2, lambda jj: (dup2[0:64, jj * KT:(jj + 1) * KT] if jj % 2 == 0
                                else qkt2[64:128, jj * KT:(jj + 1) * KT]),
                    lambda jj: (qkt2[0:64, :] if jj % 2 == 0 else dup2[64:128, :])),
            ], work_q)
            # leftovers (small s): emit before the next q_super needs them
            while work_q:
                work_q.pop(0)()

        # software-pipelined: qkv(it+1) units + deferred epilogue units
        # interleaved into attend(it). Epilogues are pushed into the LATE
        # (ACT-bound) supertiles where PE has idle slots.
        EPI_SCHED = {4: [0], 5: [1, 2], 6: [3, 4], 7: [5, 6]}
        EPI_TAIL = [7]
        if not do_attn or not k_pv:
            EPI_SCHED, EPI_TAIL = {}, []
        if do_qkv:
            for u in qkv_units(0):
                u()
        for it in range(NQS):
            work_q = []
            for es in EPI_SCHED.get(it, []):
                work_q += epilogue_units(es)
            if do_qkv and it + 1 < NQS:
                work_q += qkv_units(it + 1)
            if do_attn:
                attend_all(it, work_q)
            else:
                while work_q:
                    work_q.pop(0)()
        for es in EPI_TAIL:
            for u in epilogue_units(es, tail=True):
                u()

        main_scope.close()

        if not do_proj:
            nc.sync.dma_start(out_p[0:128, :], qt01[:, 0:C])

        rep_scope.close()

    nc.compile()
    return nc


_NC_CACHE = {}


def _get_nc(with_qkv_bias: bool, repeat: int = 1, parts=('qkv', 'attn', 'proj')):
    key = (with_qkv_bias, repeat, tuple(parts))
    if key not in _NC_CACHE:
        _NC_CACHE[key] = build_nc(with_qkv_bias, repeat, parts)
    return _NC_CACHE[key]


def _prep_inputs(x, Wqkv, bqkv, Wout, bout):
    """Build the 8 per-core input maps (bf16 host-side casts)."""
    BF = mybir.dt.np(BF16)
    x = np.asarray(x, dtype=np.float32)
    Wqkv = np.asarray(Wqkv, dtype=np.float32)
    bqkv = np.asarray(bqkv, dtype=np.float32)
    Wout = np.asarray(Wout, dtype=np.float32)

    with_qkv_bias = bool(np.any(bqkv))

    # triangular block mask: tri[kk, qq] = qq >= kk
    kk = np.arange(128)[:, None]
    qq = np.arange(128)[None, :]
    tri = (qq >= kk).astype(np.float32)

    xts = [np.ascontiguousarray(x[b].T).astype(BF) for b in range(B)]  # [C, T]

    in_maps = []
    for core in range(NCORES):
        b = core // GPB
        hs = [(core % GPB) * HPC + i for i in range(HPC)]  # 3 head indices

        def col(i, h):  # Wqkv column block for (q/k/v i, head h)
            return Wqkv[:, i * C + h * D: i * C + (h + 1) * D]

        def bias(i, h):
            return bqkv[i * C + h * D: i * C + (h + 1) * D]

        # packs: [q0|q1], [k0|k1], [q2|k2]
        wqk = np.concatenate(
            [col(0, hs[0]), col(0, hs[1]),
             col(1, hs[0]), col(1, hs[1]),
             col(0, hs[2]), col(1, hs[2])], axis=1,
        )  # [768, 384]
        wqk_c = wqk.reshape(NCH, 128, 384).transpose(1, 0, 2).reshape(128, NCH * 384)
        wqk_b = np.concatenate(
            [bias(0, hs[0]), bias(0, hs[1]),
             bias(1, hs[0]), bias(1, hs[1]),
             bias(0, hs[2]), bias(1, hs[2])]
        ).reshape(1, 384)

        # V: blocks of 65 cols per head [v_h | 0]; ones col comes from memset
        wv = np.zeros((C, VW), dtype=np.float32)
        wv_b = np.zeros((1, VW), dtype=np.float32)
        for i, h in enumerate(hs):
            wv[:, i * 65: i * 65 + 64] = col(2, h)
            wv_b[0, i * 65: i * 65 + 64] = bias(2, h)
        wv_c = wv.reshape(NCH, 128, VW).transpose(1, 0, 2).reshape(128, NCH * VW)

        # out-proj rows: h0 0-63, h1 64-127, h2 128-191
        wout_c = np.concatenate(
            [Wout[h * D:(h + 1) * D, :] for h in hs], axis=0)  # [192, 768]

        in_maps.append({
            "xt": np.ascontiguousarray(xts[b]),
            "wqk": np.ascontiguousarray(wqk_c).astype(BF),
            "wqk_b": np.ascontiguousarray(wqk_b).astype(BF),
            "wv": np.ascontiguousarray(wv_c).astype(BF),
            "wv_b": np.ascontiguousarray(wv_b).astype(BF),
            "wout": np.ascontiguousarray(wout_c).astype(BF),
            "trid": tri.astype(BF),
            "ones_d": np.ones((1, QS), dtype=np.float32).astype(BF),
        })
    return in_maps, with_qkv_bias


def kernel(x, Wqkv, bqkv, Wout, bout, _trace=False, _trace_kwargs=None, _repeat=1,
           _parts=('qkv', 'attn', 'proj')):
    in_maps, with_qkv_bias = _prep_inputs(x, Wqkv, bqkv, Wout, bout)
    nc = _get_nc(with_qkv_bias, _repeat, _parts)
    res = run_bass_kernel_spmd(
        nc, in_maps, list(range(NCORES)), trace=_trace,
        **(_trace_kwargs or {}),
    )
    bout = np.asarray(bout, dtype=np.float32)
    parts = np.stack([res.results[i]["out_p"].astype(np.float32)
                      for i in range(NCORES)])
    out = parts.reshape(B, GPB, T, C).sum(axis=1) + bout
    kernel._last_result = res
    return out.astype(np.float32)
